# revision 4
# baseline (speedup 1.0000x reference)
"""Trainium2 Bass kernel for Swin-style windowed attention.

Problem: x[64,196,768] -> qkv proj -> 12-head attention with relative
position bias -> out proj.  Sharded data-parallel over batch: 8 batch
items per NeuronCore across 8 cores.  All matmuls bf16 with fp32 PSUM
accumulation (measured rel err ~4e-3 vs the fp32 reference).

Per-core design (8 batch items):
 - QKV projection: q,k feature-major ([feat, tok] so each head's 64-dim
   slice sits on partitions), v token-major ([tok, head, 64]).  x/q/k
   are tiled per 392-token chunk; weights stored t-major ([p, t, kc, c])
   and wq/wk DMA'd in two t-slices each so the first accumulation chain
   starts as soon as x_n0 + the first weight slice land.
 - Attention in the S^T layout, head pairs processed together:
   S^T[j,i] = sum_d k[d,j] q[d,i] (K=64, the pair row-packed into the
   128-deep PE array).  The pair's two S^T tiles live in ONE two-bank
   psum tile [jlen, 2, 512] so a single strided ACT exp covers both
   heads.  Softmax runs along partitions (no max-subtract: logits are
   O(1) by construction).
 - Relative position bias applied multiplicatively after exp:
   pt = exp(S^T) * exp(rpb^T), with exp(rpb^T) precomputed on host; the
   multiply runs on the otherwise-idle GpSimd engine for the 128-row
   chunk and on DVE for the 68-row chunk.
 - PV with free softmax sums: per head ONE accumulation chain
   po_h[128, 196] with lhsT = [ones(64) | v_h(64)] -- rows 0:64 are the
   softmax sums (ones-columns replicate the column sum of P^T across 64
   partitions), rows 64:128 are O_h^T.  This removes the separate
   row-sum matmuls entirely (same streamed columns as plain PV).  The
   ones columns are interleaved into the v tiles ([tok, head, 0:64] =
   1.0, memset once at kernel start on idle engines).  Normalization:
   base-0 DVE reciprocal of rows 0:64, then partition-shifted DVE
   multiplies (in0 psum@64, out@0/64) -- verified on HW; note custom-DVE
   (reciprocal) and ACT ops silently ignore non-zero input base
   partitions, so all reciprocals read base 0.
 - Output projection contracts head pairs (K=128) accumulating over 6
   pairs with the two 384-wide halves' chains interleaved (weight loads
   hide under matmuls).  Bias b_eff = proj_bias + proj_weight @ v_bias
   (v_bias commutes through softmax-normalized attention) is added
   during the PSUM->SBUF move; q_bias*scale is folded into the q
   PSUM->SBUF copy (per-partition ACT bias); scale folded into wq on
   host.
"""

import numpy as np
import ml_dtypes

import concourse.bass as bass
import concourse.mybir as mybir
from concourse.bacc import Bacc
from concourse.bass_utils import run_bass_kernel_spmd
from concourse.tile import TileContext

F32 = mybir.dt.float32
BF16 = mybir.dt.bfloat16
AF = mybir.ActivationFunctionType
ALU = mybir.AluOpType

N_CORES = 8
B, NTOK, DIM = 64, 196, 768
H, HD = 12, 64
NHP = H // 2          # head pairs
BPC = B // N_CORES    # batches per core
TPC = BPC * NTOK      # tokens per core (1568)
SCALE = HD ** -0.5
KC = DIM // 128       # contraction chunks for 768 (6)
NT = DIM // 128       # output t-chunks (6)
TOKC = [(0, 128), (128, 68)]   # token chunking of 196
NQCH = 4              # token N-chunks (1568/392); 392 = 2 batches
NQW = TPC // NQCH     # 392


def build_nc():
    nc = Bacc()

    # x_t2[p, n, kc, w] = x_fm[kc*128+p, n*392+w] -> per-partition contiguous
    x_t = nc.declare_dram_parameter("x_t", [128, NQCH, KC, NQW], BF16, False)
    # w_all[p, w, t, kc, c] = W_w^T[kc*128+p, t*128+c] for w in (q,k,v,p)
    w_all = nc.declare_dram_parameter("w_all", [128, 4, NT, KC, 128], BF16, False)
    qb = nc.declare_dram_parameter("qb", [128, KC], F32, False)
    beff = nc.declare_dram_parameter("beff", [128, DIM], F32, False)
    # exp(rpb^T), bf16, split by ktok chunk: [j, head, i]
    rpb0 = nc.declare_dram_parameter("rpb0", [128, H, NTOK], BF16, False)
    rpb1 = nc.declare_dram_parameter("rpb1", [68, H, NTOK], BF16, False)
    y = nc.declare_dram_parameter("y", [TPC, DIM], F32, True)

    with TileContext(nc) as tc, \
         tc.tile_pool(name="const", bufs=1) as cpool:
        def ctile(shape, dtype, nm):
            return cpool.tile(shape, dtype, name=nm, tag=nm)

        # ---------------- inputs (DMA ordered by first use) ----------------
        x_n = [ctile([128, KC, NQW], BF16, f"xn{n}") for n in range(NQCH)]
        w_t = [ctile([128, NT, KC, 128], BF16, f"wt{w}") for w in range(4)]
        x_sb = {(kc, n): x_n[n][:, kc, :]
                for kc in range(KC) for n in range(NQCH)}

        # v tiles: [tok, head, 128] with cols 0:64 = ones, 64:128 = v_h.
        # lhsT for PV head h is v_sb[:, h, :] = [ones | v_h].
        v_sb = {}
        for b in range(BPC):
            for ci, (toff, tlen) in enumerate(TOKC):
                v_sb[(b, ci)] = ctile([tlen, H, 128], BF16, f"v{b}_{ci}")
        # memset the ones columns first: no input deps, runs while DMAs land
        for i, (b, ci) in enumerate(sorted(v_sb)):
            eng = (nc.gpsimd, nc.vector)[i % 2]
            eng.memset(v_sb[(b, ci)][:, :, 0:64], 1.0)

        nc.sync.dma_start(x_n[0][:], x_t[:, 0])
        # wq/wk in two t-slices each so t=0 chains start early
        nc.sync.dma_start(w_t[0][:, 0:2], w_all[:, 0, 0:2])
        nc.sync.dma_start(w_t[0][:, 2:NT], w_all[:, 0, 2:NT])
        nc.sync.dma_start(w_t[1][:, 0:2], w_all[:, 1, 0:2])
        nc.sync.dma_start(w_t[1][:, 2:NT], w_all[:, 1, 2:NT])
        qb_sb = ctile([128, KC], F32, "qb_sb")
        nc.sync.dma_start(qb_sb[:], qb[:])
        rpb0_sb = ctile([128, H, NTOK], BF16, "rpb0_sb")
        nc.sync.dma_start(rpb0_sb[:], rpb0[:])
        rpb1_sb = ctile([68, H, NTOK], BF16, "rpb1_sb")
        nc.sync.dma_start(rpb1_sb[:], rpb1[:])
        nc.sync.dma_start(w_t[2][:], w_all[:, 2])
        for n in range(1, NQCH):
            nc.sync.dma_start(x_n[n][:], x_t[:, n])
        nc.sync.dma_start(w_t[3][:], w_all[:, 3])
        beff_bc = ctile([128, DIM], F32, "beff_bc")
        nc.sync.dma_start(beff_bc[:], beff[:])

        # ---------------- persistent activations ----------------
        q_sb = {(t, n): ctile([128, NQW], BF16, f"q{t}_{n}")
                for t in range(NT) for n in range(NQCH)}
        k_sb = {(t, n): ctile([128, NQW], BF16, f"k{t}_{n}")
                for t in range(NT) for n in range(NQCH)}

        # ---------------- phase A: QKV projection ----------------
        with tc.tile_pool(name="qkv_ps", bufs=4, space="PSUM") as pqk:
            for n in range(NQCH):
                for t in range(NT):
                    psq = pqk.tile([128, NQW], F32, tag="qkps")
                    for kc in range(KC):
                        nc.tensor.matmul(
                            psq[:], w_t[0][:, t, kc, :],
                            x_sb[(kc, n)][:], start=(kc == 0),
                            stop=(kc == KC - 1))
                    nc.scalar.activation(q_sb[(t, n)][:], psq[:],
                                         AF.Identity, bias=qb_sb[:, t:t + 1])
                    psk = pqk.tile([128, NQW], F32, tag="qkps")
                    for kc in range(KC):
                        nc.tensor.matmul(
                            psk[:], w_t[1][:, t, kc, :],
                            x_sb[(kc, n)][:], start=(kc == 0),
                            stop=(kc == KC - 1))
                    nc.scalar.activation(k_sb[(t, n)][:], psk[:], AF.Copy)
                for b in (2 * n, 2 * n + 1):
                    for ci, (toff, tlen) in enumerate(TOKC):
                        c0 = (b % 2) * NTOK + toff
                        for nh in range(2):
                            psv = pqk.tile([128, 384], F32, tag="vps")
                            for kc in range(KC):
                                nc.tensor.matmul(
                                    psv[:tlen], x_sb[(kc, n)][:, c0:c0 + tlen],
                                    w_t[2][:, 3 * nh:3 * nh + 3, kc, :],
                                    start=(kc == 0), stop=(kc == KC - 1))
                            nc.scalar.activation(
                                v_sb[(b, ci)][:, 6 * nh:6 * nh + 6, 64:128],
                                psv[:tlen]
                                .rearrange("p (a b) -> p a b", a=6),
                                AF.Copy)

        # ---------------- phase B: attention + out projection ----------------
        with tc.tile_pool(name="s_ps", bufs=1, space="PSUM") as ps_s, \
             tc.tile_pool(name="o_ps", bufs=1, space="PSUM") as ps_o, \
             tc.tile_pool(name="proj_ps", bufs=2, space="PSUM") as ps_proj, \
             tc.tile_pool(name="pr_sbuf", bufs=4) as praw_pool, \
             tc.tile_pool(name="p_sbuf", bufs=4) as p_pool, \
             tc.tile_pool(name="r_sbuf", bufs=3) as r_pool, \
             tc.tile_pool(name="o_sbuf", bufs=14) as o_pool, \
             tc.tile_pool(name="y_sbuf", bufs=6) as y_pool:
            o_tiles = {}

            def stage1(b, hp):
                """S^T matmuls + exp + rpbE multiply -> p_tiles dict."""
                n = b // 2
                q0 = (b % 2) * NTOK
                p_tiles = {}
                for ci, (joff, jlen) in enumerate(TOKC):
                    rpb_sb = rpb0_sb if ci == 0 else rpb1_sb
                    jsl = slice(q0 + joff, q0 + joff + jlen)
                    rpb_pair = rpb_sb[:jlen, 2 * hp:2 * hp + 2, :] \
                        .rearrange("p h n -> p (h n)")
                    pt = p_pool.tile([jlen, 2 * NTOK], BF16, tag=f"p{ci}")
                    praw = praw_pool.tile([jlen, 2 * NTOK], BF16,
                                          tag=f"pr{ci}")
                    # [jlen, 2, 512] = one PSUM bank per head half; a single
                    # strided ACT exp then covers both heads in one op.
                    pss = ps_s.tile([jlen, 2, 512], F32, tag=f"s{ci}")
                    for hh in range(2):
                        rows = slice(hh * 64, hh * 64 + 64)
                        nc.tensor.matmul(
                            pss[:, hh, 0:NTOK], k_sb[(hp, n)][rows, jsl],
                            q_sb[(hp, n)][rows, q0:q0 + NTOK],
                            start=True, stop=True)
                    nc.scalar.activation(
                        praw[:].rearrange("p (a b) -> p a b", a=2),
                        pss[:, :, 0:NTOK], AF.Exp)
                    eng = nc.gpsimd if ci == 0 else nc.vector
                    eng.tensor_tensor(pt[:], praw[:], rpb_pair, ALU.mult)
                    p_tiles[ci] = pt
                return p_tiles

            def stage2(b, hp, p_tiles):
                """Per-head PV with free sums, reciprocal, normalize.

                po_h rows 0:64 = softmax sums (from the ones columns of
                v_sb), rows 64:128 = O_h^T.  Full-bank tiles so the two
                heads' accumulation chains never share a PSUM bank.
                """
                po = ps_o.tile([128, 2, 512], F32, tag="oh")
                for ci, (joff, jlen) in enumerate(TOKC):
                    for hh in range(2):
                        nc.tensor.matmul(
                            po[:, hh, 0:NTOK], v_sb[(b, ci)][:, 2 * hp + hh, :],
                            p_tiles[ci][:, hh * NTOK:(hh + 1) * NTOK],
                            start=(ci == 0), stop=(ci == 1))
                rbc = r_pool.tile([64, 2, NTOK], F32, tag="rbc")
                for hh in range(2):
                    nc.vector.reciprocal_approx_fast(
                        out=rbc[:, hh, :], in_=po[0:64, hh, 0:NTOK])
                ot = o_pool.tile([128, NTOK], BF16, tag="o_sb")
                for hh in range(2):
                    nc.vector.tensor_tensor(
                        ot[hh * 64:hh * 64 + 64, :],
                        po[64:128, hh, 0:NTOK], rbc[:, hh, :], ALU.mult)
                o_tiles[(b, hp)] = ot

            def proj(b):
                for ci, (toff, tlen) in enumerate(TOKC):
                    psy = [ps_proj.tile([128, 384], F32, tag="proj",
                                        name=f"psy{nh}") for nh in range(2)]
                    for hp in range(NHP):
                        for nh in range(2):
                            nc.tensor.matmul(
                                psy[nh][:tlen],
                                o_tiles[(b, hp)][:, toff:toff + tlen],
                                w_t[3][:, 3 * nh:3 * nh + 3, hp, :],
                                start=(hp == 0), stop=(hp == NHP - 1))
                    tok0 = b * NTOK + toff
                    for nh in range(2):
                        yt = y_pool.tile([128, 384], F32, tag="y")
                        nc.vector.tensor_tensor(
                            yt[:tlen], psy[nh][:tlen],
                            beff_bc[:tlen, nh * 384:(nh + 1) * 384], ALU.add)
                        nc.sync.dma_start(
                            y[tok0:tok0 + tlen, nh * 384:(nh + 1) * 384],
                            yt[:tlen])

            blocks = [(b, hp) for b in range(BPC) for hp in range(NHP)]
            for b, hp in blocks:
                p_tiles = stage1(b, hp)
                stage2(b, hp, p_tiles)
                if hp == NHP - 1:
                    proj(b)
    nc.finalize()
    return nc


def prep_host(x, qkv_weight, q_bias, v_bias, rpb_table, rel_pos_index,
              proj_weight, proj_bias):
    """Host-side prep: transposes, dtype casts, bias folding, rpb gather."""
    bf16 = ml_dtypes.bfloat16
    x = np.asarray(x, np.float32)
    qkv_weight = np.asarray(qkv_weight, np.float32)
    proj_weight = np.asarray(proj_weight, np.float32)
    q_bias = np.asarray(q_bias, np.float32)
    v_bias = np.asarray(v_bias, np.float32)
    rpb_table = np.asarray(rpb_table, np.float32)
    rel_pos_index = np.asarray(rel_pos_index)
    proj_bias = np.asarray(proj_bias, np.float32)

    # scale folded into q projection weights + bias
    wq = qkv_weight[0:DIM].T * SCALE
    wk = qkv_weight[DIM:2 * DIM].T
    wv = qkv_weight[2 * DIM:3 * DIM].T
    wp = proj_weight.T
    # w_all[p, w, t, kc, c] = W_w^T[kc*128+p, t*128+c]
    w_all = np.stack(
        [w.reshape(KC, 128, NT, 128) for w in (wq, wk, wv, wp)],
        axis=0)                                    # [4, kc, p, t, c]
    w_all = np.ascontiguousarray(
        w_all.transpose(2, 0, 3, 1, 4)).astype(bf16)  # [p, 4, t, kc, c]
    qb = np.ascontiguousarray((q_bias * SCALE).reshape(KC, 128).T).astype(np.float32)
    beff = np.ascontiguousarray(np.broadcast_to(
        (proj_bias + proj_weight @ v_bias).reshape(1, DIM), (128, DIM))).astype(np.float32)

    rpb_full = rpb_table[rel_pos_index.reshape(-1)].reshape(NTOK, NTOK, H)
    rpbT = np.exp(np.ascontiguousarray(rpb_full.transpose(1, 2, 0)),
                  dtype=np.float32)
    rpb0 = np.ascontiguousarray(rpbT[0:128]).astype(bf16)
    rpb1 = np.ascontiguousarray(rpbT[128:NTOK]).astype(bf16)

    shared = dict(w_all=w_all, qb=qb, beff=beff, rpb0=rpb0, rpb1=rpb1)
    in_maps = []
    for c in range(N_CORES):
        xs = x[c * BPC:(c + 1) * BPC]                       # [8,196,768]
        x_fm = xs.transpose(2, 0, 1).reshape(DIM, TPC)      # [768, 1568]
        # x_t2[p, n, kc, w] = x_fm[kc*128+p, n*392+w]
        x_tc = np.ascontiguousarray(
            x_fm.reshape(KC, 128, NQCH, NQW).transpose(1, 2, 0, 3)
        ).astype(bf16)
        in_maps.append(dict(shared, x_t=x_tc))
    return in_maps


_NC_CACHE = {}


def get_nc():
    if "nc" not in _NC_CACHE:
        _NC_CACHE["nc"] = build_nc()
    return _NC_CACHE["nc"]


def kernel(**inputs):
    nc = get_nc()
    in_maps = prep_host(**inputs)
    res = run_bass_kernel_spmd(nc, in_maps, list(range(N_CORES)))
    outs = [res.results[c]["y"].reshape(BPC, NTOK, DIM) for c in range(N_CORES)]
    return np.concatenate(outs, axis=0).astype(np.float32)
